# revision 14
# baseline (speedup 1.0000x reference)
"""Trainium2 Bass kernel for nn_MetricSelfAttention.

Reference computation (B=4, W=2048, C=1024, N=16 heads, K=64):
    metric_n = P_n @ P_n^T                  (per-head bilinear form)
    proj = X @ W_proj^T ; split into per-head Q_n [W, K]
    S_n = tril(Q_n M_n Q_n^T) / sqrt(K)     (multiplicative causal mask, no softmax)
    U_n = S_n @ Q_n
    out = concat_n(U_n @ T_n) @ W_mixer^T

Device algorithm (per core; 8 cores = 4 batches x 2 head-groups of 8 heads):
  Host folds:  M'_n = P_n P_n^T / sqrt(K),  Wm2_n = T_n @ W_mixer[:, nK:(n+1)K]^T
  so that out_partial = sum_n U_n @ Wm2_n with U_n = tril(Q_n M'_n Q_n^T) @ Q_n.

  Causal decomposition (block row i of 128):
    U_i = A_i @ KV_i + tril(A_i Q_i^T) @ Q_i,   A = Q M',  KV_i = sum_{j<i} Q_j^T Q_j

  v3 structure:
   - projection and mixer run in fp8e4 DoubleRow (2 contraction k-tiles per
     matmul at 0.5 cycles/row) using a 3-term residual split
     X@W ~= X8@W8 + R8@W8 + X8@RW8 (R8/RW8 = fp8 residuals at natural scale),
     which restores bf16-level accuracy at 0.75x the bf16 PE cost
   - U is produced at 0.25 scale (folded into the A drain) so it fits fp8e4
     range; the mixer consumes u8 + ur8 (residual) against host-split
     wm8/wmr8 (x64), with the net 4/64 scale folded into the output drain
   - the transposed projection QT comes from PE transposes of the natural
     projection; PSUM drains are batched [128,512] ops spread across
     DVE/Scalar; the KV prefix is 4 per-pair add chains on GpSimd
"""

import os
import sys

import numpy as np
import ml_dtypes

if "/opt/trn_rl_repo" not in sys.path:
    sys.path.insert(0, "/opt/trn_rl_repo")

import concourse.bass as bass
import concourse.tile as tile
from concourse import bacc, mybir
from concourse.bass_utils import run_bass_kernel_spmd

BF16 = mybir.dt.bfloat16
F32 = mybir.dt.float32
F8 = mybir.dt.float8e4
DR = mybir.MatmulPerfMode.DoubleRow

B, W, C, NHEADS, K = 4, 2048, 1024, 16, 64
HPG = 8          # heads per group (per core)
NPAIR = 4        # head pairs per core
GK = HPG * K     # 512: head-group projection width
WP_SCALE = 32.0  # fp8 pre-scale on W_proj (values ~0.02 -> ~0.64)
WM_SCALE = 64.0  # fp8 pre-scale on the folded mixer weights
U_SCALE = 0.25   # U pre-scale (via the A drain) so 0.25*U fits fp8e4 range

_NC_CACHE = {}
LAST_RESULTS = None  # for test.py introspection (exec_time_ns etc.)


def build_nc(w=W):
    """Build the per-core Bass program. Parameterized by sequence length for
    small-scale simulator testing."""
    nw = w // 128           # number of 128-row w-tiles
    chunk = min(512, w)
    nch = w // chunk        # 512-wide chunks of the sequence dim

    nc = bacc.Bacc()
    x8_d = nc.declare_dram_parameter("x8", [4, 128, 2, w], F8, isOutput=False)
    r8_d = nc.declare_dram_parameter("r8", [4, 128, 2, w], F8, isOutput=False)
    wp8_d = nc.declare_dram_parameter("wp8", [4, 128, 2, GK], F8, isOutput=False)
    rw8_d = nc.declare_dram_parameter("rw8", [4, 128, 2, GK], F8, isOutput=False)
    mblk_d = nc.declare_dram_parameter("mblk", [NPAIR, 128, 128], BF16, isOutput=False)
    wmA_d = nc.declare_dram_parameter("wmA", [NPAIR, 128, 2, C], F8, isOutput=False)
    wmB_d = nc.declare_dram_parameter("wmB", [2, 128, 2, C], F8, isOutput=False)
    triu4_d = nc.declare_dram_parameter("triu4", [128, 512], F32, isOutput=False)
    blkd4_d = nc.declare_dram_parameter("blkd4", [128, 512], F32, isOutput=False)
    ident_d = nc.declare_dram_parameter("ident", [128, 128], BF16, isOutput=False)
    out_d = nc.declare_dram_parameter("out", [w, C], F32, isOutput=True)

    from contextlib import ExitStack

    with tile.TileContext(nc) as tc, ExitStack() as ctx:
        const = ctx.enter_context(tc.tile_pool(name="const", bufs=1))
        persist = ctx.enter_context(tc.tile_pool(name="persist", bufs=1))

        # ---- input loads, spread across issue queues ----
        # sync: x8/r8 chunk-major so phase A starts after the first chunk.
        # scalar gets ident (needed by the first transpose) + some weights;
        # gpsimd the rest.  Descriptor issue is ~650ns/op on one sequencer.
        x8_sb = [const.tile([128, 2, w], F8, name=f"x8_{j}", tag=f"x8_{j}")
                 for j in range(4)]
        r8_sb = [const.tile([128, 2, w], F8, name=f"r8_{j}", tag=f"r8_{j}")
                 for j in range(4)]
        for ch in range(nch):
            for j in range(4):
                nc.sync.dma_start(
                    x8_sb[j][:, :, chunk * ch:chunk * (ch + 1)],
                    x8_d[j][:, :, chunk * ch:chunk * (ch + 1)],
                )
            for j in range(4):
                nc.sync.dma_start(
                    r8_sb[j][:, :, chunk * ch:chunk * (ch + 1)],
                    r8_d[j][:, :, chunk * ch:chunk * (ch + 1)],
                )
        wp8_sb = []
        rw8_sb = []
        for j in range(4):
            t = const.tile([128, 2, GK], F8, name=f"wp8_{j}", tag=f"wp8_{j}")
            nc.gpsimd.dma_start(t[:], wp8_d[j])
            wp8_sb.append(t)
        for j in range(4):
            t = const.tile([128, 2, GK], F8, name=f"rw8_{j}", tag=f"rw8_{j}")
            (nc.gpsimd if j < 2 else nc.scalar).dma_start(t[:], rw8_d[j])
            rw8_sb.append(t)
        ident_sb = const.tile([128, 128], BF16, name="ident", tag="ident")
        nc.scalar.dma_start(ident_sb[:], ident_d[:])
        mblk_sb = const.tile([128, NPAIR * 128], BF16, name="mblk", tag="mblk")
        for p in range(NPAIR):
            nc.scalar.dma_start(mblk_sb[:, 128 * p:128 * (p + 1)], mblk_d[p])
        wmA_sb = []
        for p in range(NPAIR):
            t = const.tile([128, 2, C], F8, name=f"wmA_{p}", tag=f"wmA_{p}")
            nc.gpsimd.dma_start(t[:], wmA_d[p])
            wmA_sb.append(t)
        wmB_sb = []
        for q in range(2):
            t = const.tile([128, 2, C], F8, name=f"wmB_{q}", tag=f"wmB_{q}")
            nc.gpsimd.dma_start(t[:], wmB_d[q])
            wmB_sb.append(t)
        triu4_sb = const.tile([128, 512], F32, name="triu4", tag="triu4")
        nc.gpsimd.dma_start(triu4_sb[:], triu4_d[:])
        blkd4_sb = const.tile([128, 512], F32, name="blkd4", tag="blkd4")
        nc.gpsimd.dma_start(blkd4_sb[:], blkd4_d[:])

        # ---- persistent intermediates ----
        # q_nat: natural layout [w, k] -- w-tile i occupies cols [512i, 512i+512),
        #        inside which head h (0..7) owns cols [64h, 64h+64).
        q_nat = persist.tile([128, nw * GK], BF16, name="q_nat", tag="q_nat")
        # qt/at: transposed layout per pair p: cols [p*w, (p+1)*w); partitions
        #        0-63 = head 2p's K dims, 64-127 = head 2p+1's.
        qt_sb = persist.tile([128, NPAIR * w], BF16, name="qt_sb", tag="qt_sb")
        at_sb = persist.tile([128, NPAIR * w], BF16, name="at_sb", tag="at_sb")
        # per-i blockdiag(KV_a, KV_b) lhsT tiles, layout [p][i]
        kv_sb = persist.tile([128, nw * NPAIR * 128], BF16, name="kv_sb",
                             tag="kv_sb")
        # pre-masked gram terms, layout [i][p]
        gt_all = persist.tile([128, max(nw - 1, 1) * NPAIR * 128], BF16,
                              name="gt_all", tag="gt_all")
        st_all = persist.tile([128, nw * NPAIR * 256], BF16, name="st_all",
                              tag="st_all")
        # mixer lhsT storage: per (i, p) 256 cols = [u8 | ur8] fp8
        u_mix = persist.tile([128, nw * NPAIR * 256], F8, name="u_mix",
                             tag="u_mix")

        # ============ phase 1: projection + transposes + gram + C ============
        with tc.tile_pool(name="psA", bufs=3, space="PSUM") as psA, \
                tc.tile_pool(name="psT", bufs=2, space="PSUM") as psT, \
                tc.tile_pool(name="psG", bufs=2, space="PSUM") as psG:

            def emit_A(i):
                # 3-term fp8 DoubleRow projection: X8 W8 + R8 W8 + X8 RW8,
                # accumulated in one psum tile, drained with the 1/32 unscale.
                ps = psA.tile([128, GK], F32, name="projnat", tag="projnat")
                terms = [(x8_sb, wp8_sb), (r8_sb, wp8_sb), (x8_sb, rw8_sb)]
                for t, (lhs, rhs) in enumerate(terms):
                    for j in range(4):
                        nc.tensor.matmul(
                            ps[:],
                            lhsT=lhs[j][:, :, 128 * i:128 * (i + 1)],
                            rhs=rhs[j][:],
                            start=(t == 0 and j == 0),
                            stop=(t == 2 and j == 3),
                            perf_mode=DR,
                        )
                nc.vector.tensor_scalar_mul(
                    q_nat[:, GK * i:GK * (i + 1)], ps[:], 1.0 / WP_SCALE
                )

            def emit_T(i):
                # transpose the 4 pair-blocks of q_nat tile i into one psum
                # tile, then one scalar copy into the strided qt_sb layout
                # ([128 part, pair (stride w), 128 w-cols])
                ps = psT.tile([128, 512], BF16, name="qtT", tag="qtT")
                for p in range(NPAIR):
                    nc.tensor.transpose(
                        ps[:, 128 * p:128 * (p + 1)],
                        q_nat[:, GK * i + 128 * p:GK * i + 128 * (p + 1)],
                        ident_sb[:],
                    )
                qt_view = qt_sb[:].rearrange(
                    "part (n wdim) -> part n wdim", n=NPAIR
                )[:, :, 128 * i:128 * (i + 1)]
                nc.scalar.copy(qt_view, ps[:])

            def emit_G(i):
                # gram term for w-tile i (pair-stacked), blockdiag-masked on
                # drain straight to bf16 (GpSimd can't touch PSUM -> DVE)
                ps = psG.tile([128, NPAIR * 128], F32, name="gterm", tag="gterm")
                for p in range(NPAIR):
                    qp = q_nat[:, GK * i + 128 * p:GK * i + 128 * (p + 1)]
                    nc.tensor.matmul(
                        ps[:, 128 * p:128 * (p + 1)],
                        lhsT=qp, rhs=qp,
                        start=(p == 0),
                        stop=(p == NPAIR - 1),
                    )
                nc.vector.tensor_mul(
                    gt_all[:, i * NPAIR * 128:(i + 1) * NPAIR * 128],
                    ps[:], blkd4_sb[:],
                )

            def emit_C(ch):
                # AT_pair = blockdiag(M'a, M'b) @ QT_pair, drained at 0.25
                # scale (U prescale) on the scalar engine
                for p in range(NPAIR):
                    ps = psA.tile([128, chunk], F32, name="atps", tag="projnat")
                    nc.tensor.matmul(
                        ps[:],
                        lhsT=mblk_sb[:, 128 * p:128 * (p + 1)],
                        rhs=qt_sb[:, p * w + chunk * ch:p * w + chunk * (ch + 1)],
                        start=True,
                        stop=True,
                    )
                    nc.scalar.mul(
                        at_sb[:, p * w + chunk * ch:p * w + chunk * (ch + 1)],
                        ps[:], U_SCALE,
                    )

            for i in range(nw):
                emit_A(i)
                if i >= 1:
                    emit_T(i - 1)
                    if i - 1 < nw - 1:
                        emit_G(i - 1)
                if i >= 1 and i % 4 == 0:
                    emit_C(i // 4 - 1)
            emit_T(nw - 1)
            emit_C(nch - 1)

        # ---- KV prefix: 4 independent per-pair bf16 add chains ----
        # kv[p, 0] unused (i=0 has no main term).  All-SBUF work on GpSimd,
        # keeping DVE/Scalar free for PSUM drains.
        for p in range(NPAIR):
            nc.gpsimd.tensor_copy(
                kv_sb[:, (p * nw + 1) * 128:(p * nw + 1) * 128 + 128],
                gt_all[:, 128 * p:128 * (p + 1)],
            )
            for i in range(2, nw):
                nc.gpsimd.tensor_add(
                    kv_sb[:, (p * nw + i) * 128:(p * nw + i) * 128 + 128],
                    kv_sb[:, (p * nw + i - 1) * 128:(p * nw + i - 1) * 128 + 128],
                    gt_all[:, (i - 1) * NPAIR * 128 + 128 * p:(i - 1) * NPAIR * 128 + 128 * (p + 1)],
                )

        # ============ phase 2/3: D1a + D1b + mixer, interleaved per i ========
        with tc.tile_pool(name="psS", bufs=4, space="PSUM") as psS, \
                tc.tile_pool(name="psU", bufs=2, space="PSUM") as psU, \
                tc.tile_pool(name="psM", bufs=2, space="PSUM") as psM, \
                tc.tile_pool(name="outp", bufs=3) as outp:

            def emit_D1a(i):
                # all 8 diagonal blocks S_ii^T = Q_i @ A_i^T of tile i.
                # The h=0 / h=1 matmuls use contraction row groups [0:64) /
                # [64:128) and can execute CONCURRENTLY in the PE array, so
                # they must land in different PSUM banks: batch by h.
                for h in range(2):
                    ps = psS.tile([128, 512], F32, name="st", tag="st")
                    for p in range(NPAIR):
                        nc.tensor.matmul(
                            ps[:, 128 * p:128 * (p + 1)],
                            lhsT=qt_sb[64 * h:64 * (h + 1),
                                       p * w + 128 * i:p * w + 128 * (i + 1)],
                            rhs=at_sb[64 * h:64 * (h + 1),
                                      p * w + 128 * i:p * w + 128 * (i + 1)],
                            start=(p == 0),
                            stop=(p == NPAIR - 1),
                        )
                    # masked drain into the strided st_all layout
                    # (cols i*1024 + p*256 + h*128)
                    dst = st_all[:, i * NPAIR * 256:(i + 1) * NPAIR * 256].rearrange(
                        "part (p two) -> part p two", p=NPAIR
                    )[:, :, 128 * h:128 * (h + 1)]
                    nc.vector.tensor_mul(dst, ps[:], triu4_sb[:])

            def emit_D1b(i):
                # UT for all 4 pairs of tile i in one [128, 512] psum tile.
                # Zero regions are per-partition 2KB rows, so the full-width
                # kv matmuls carry the visible start/stop bookkeeping; the
                # partition-split diag matmuls are inexpressible to the sim's
                # flat group tracker (HW has_written bits are per partition)
                # and use skip_group_check, with start=True only on the first
                # write to each partition-row range (i==0, p==0).
                ps = psU.tile([128, 512], F32, name="ut", tag="ut")
                for p in range(NPAIR):
                    st0 = i * NPAIR * 256 + 256 * p
                    if i > 0:
                        nc.tensor.matmul(
                            ps[:, 128 * p:128 * (p + 1)],
                            lhsT=kv_sb[:, (p * nw + i) * 128:(p * nw + i) * 128 + 128],
                            rhs=at_sb[:, p * w + 128 * i:p * w + 128 * (i + 1)],
                            start=(p == 0),
                            stop=(p == NPAIR - 1),
                        )
                    for h in range(2):
                        nc.tensor.matmul(
                            ps[64 * h:64 * (h + 1), 128 * p:128 * (p + 1)],
                            lhsT=q_nat[:, GK * i + 128 * p + 64 * h:
                                       GK * i + 128 * p + 64 * (h + 1)],
                            rhs=st_all[:, st0 + 128 * h:st0 + 128 * (h + 1)],
                            start=(i == 0 and p == 0),
                            stop=(i == 0 and p == NPAIR - 1 and h == 1),
                            skip_group_check=True,
                        )
                # drains: u8 = fp8(0.25 U) on scalar, ur8 = fp8 residual on DVE
                seg = u_mix[:, i * NPAIR * 256:(i + 1) * NPAIR * 256].rearrange(
                    "part (p two) -> part p two", p=NPAIR
                )
                u8_view = seg[:, :, 0:128]
                ur8_view = seg[:, :, 128:256]
                nc.scalar.copy(u8_view, ps[:])
                nc.vector.tensor_sub(ur8_view, ps[:], u8_view)

            def emit_mixer(i):
                # out_i = (u8 + ur8) @ (wm8 + wmr8) via 3 fp8 DR terms:
                #   t1: pair-paired u8 x wm8   (2 matmuls)
                #   t23: per pair (u8 x wmr8 + ur8 x wm8) slot-paired (4)
                out_sb = outp.tile([128, C], F32, name="out_sb", tag="out_sb")
                for cm in range(C // 512):
                    mx = psM.tile([128, 512], F32, name="mx", tag="mx")
                    for q in range(2):
                        lhs = u_mix[:, i * NPAIR * 256 + 512 * q:
                                    i * NPAIR * 256 + 512 * (q + 1)].rearrange(
                            "part (two c) -> part two c", two=2
                        )[:, :, 0:128]
                        nc.tensor.matmul(
                            mx[:],
                            lhsT=lhs,
                            rhs=wmB_sb[q][:, :, 512 * cm:512 * (cm + 1)],
                            start=(q == 0),
                            stop=False,
                            perf_mode=DR,
                        )
                    for p in range(NPAIR):
                        lhs = u_mix[:, i * NPAIR * 256 + 256 * p:
                                    i * NPAIR * 256 + 256 * (p + 1)].rearrange(
                            "part (two c) -> part two c", two=2
                        )
                        nc.tensor.matmul(
                            mx[:],
                            lhsT=lhs,
                            rhs=wmA_sb[p][:, :, 512 * cm:512 * (cm + 1)],
                            start=False,
                            stop=(p == NPAIR - 1),
                            perf_mode=DR,
                        )
                    # psM = (U_SCALE*U) @ (WM_SCALE*Wm2)
                    scale = 1.0 / (U_SCALE * WM_SCALE)
                    if cm == 0:
                        nc.vector.tensor_scalar_mul(
                            out_sb[:, 512 * cm:512 * (cm + 1)], mx[:], scale)
                    else:
                        nc.scalar.mul(
                            out_sb[:, 512 * cm:512 * (cm + 1)], mx[:], scale)
                nc.sync.dma_start(out_d[128 * i:128 * (i + 1), :], out_sb[:])

            # pipeline: D1a runs one tile ahead of D1b/mixer
            emit_D1a(0)
            for i in range(nw):
                if i + 1 < nw:
                    emit_D1a(i + 1)
                emit_D1b(i)
                emit_mixer(i)

    nc.finalize()
    return nc


def _get_nc(w=W):
    if w not in _NC_CACHE:
        _NC_CACHE[w] = build_nc(w)
    return _NC_CACHE[w]


def _pair_split(m):
    """[C, F] -> [4, 128, 2, F]: c-pair j, slot s = c rows 256j+128s.."""
    cdim, f = m.shape
    return np.ascontiguousarray(
        m.reshape(4, 2, 128, f).transpose(0, 2, 1, 3)
    )


def make_in_maps(x, wp, pm, tf, wm, w=W):
    """Host-side shard prep: per-core input dict list (cores c: b=c%4, g=c//4)."""
    bf = ml_dtypes.bfloat16
    f8 = ml_dtypes.float8_e4m3fn
    metric = np.einsum("nij,nkj->nik", pm, pm) / np.sqrt(np.float32(K))
    # Wm2_n = T_n @ W_mixer[:, nK:(n+1)K]^T : [K, C]
    wm2 = np.stack([tf[n] @ wm[:, n * K:(n + 1) * K].T for n in range(NHEADS)])

    tri = np.triu(np.ones((128, 128), np.float32))
    triu4 = np.tile(tri, (1, 4)).astype(np.float32)
    blkd = np.zeros((128, 128), np.float32)
    blkd[:64, :64] = 1.0
    blkd[64:, 64:] = 1.0
    blkd4 = np.tile(blkd, (1, 4)).astype(np.float32)
    ident = np.eye(128, dtype=np.float32)

    in_maps = []
    for c in range(8):
        b, g = c % 4, c // 4
        xt = np.ascontiguousarray(x[b][:w].T)                    # [C, w] f32
        x8f = xt.astype(f8)
        r8f = (xt - x8f.astype(np.float32)).astype(f8)
        wpt = np.ascontiguousarray(wp[GK * g:GK * (g + 1), :].T) * WP_SCALE  # [C, GK]
        wp8f = wpt.astype(f8)
        rw8f = (wpt - wp8f.astype(np.float32)).astype(f8)

        mblk = np.zeros((NPAIR, 128, 128), np.float32)
        wm2c = np.zeros((NPAIR, 128, C), np.float32)
        for p in range(NPAIR):
            ha, hb = HPG * g + 2 * p, HPG * g + 2 * p + 1
            mblk[p, :64, :64] = metric[ha]
            mblk[p, 64:, 64:] = metric[hb]
            wm2c[p, :64, :] = wm2[ha]
            wm2c[p, 64:, :] = wm2[hb]
        wm2c *= WM_SCALE
        wm8 = wm2c.astype(f8)
        wmr8 = (wm2c - wm8.astype(np.float32)).astype(f8)
        wmA = np.stack([wmr8, wm8], axis=2)                      # [4,128,2,C]
        wmB = np.stack(
            [np.stack([wm8[2 * q], wm8[2 * q + 1]], axis=1) for q in range(2)]
        )                                                        # [2,128,2,C]

        in_maps.append({
            "x8": _pair_split(x8f),
            "r8": _pair_split(r8f),
            "wp8": _pair_split(wp8f),
            "rw8": _pair_split(rw8f),
            "mblk": mblk.astype(bf),
            "wmA": np.ascontiguousarray(wmA),
            "wmB": np.ascontiguousarray(wmB),
            "triu4": triu4,
            "blkd4": blkd4,
            "ident": ident.astype(bf),
        })
    return in_maps


def _ensure_ntff_hook():
    """The agent image lacks antenv.axon_hooks; synthesize it and register the
    ctypes NTFF profile hook from trn_agent_boot so trace=True works."""
    try:
        from antenv.axon_hooks import get_axon_ntff_profile_hook  # noqa: F401
        return
    except ImportError:
        pass
    import types

    import antenv

    mod = types.ModuleType("antenv.axon_hooks")
    _box = {}
    mod.set_axon_ntff_profile_hook = lambda h: _box.__setitem__("h", h)
    mod.get_axon_ntff_profile_hook = lambda: _box.get("h")
    sys.modules["antenv.axon_hooks"] = mod
    antenv.axon_hooks = mod
    try:
        from trn_agent_boot.trn_boot import _ntff_profile_via_ctypes

        h = _ntff_profile_via_ctypes("/opt/axon/libaxon_pjrt.so")
        if h is not None:
            mod.set_axon_ntff_profile_hook(h)
    except Exception as e:  # profiling degrades, run still works
        print(f"ntff hook setup failed: {e}", file=sys.stderr)


def kernel(**inputs):
    global LAST_RESULTS
    x = np.asarray(inputs["in_sequence_bwc"], np.float32)
    wp = np.asarray(inputs["W_proj"], np.float32)
    pm = np.asarray(inputs["pre_metric_nkk"], np.float32)
    tf = np.asarray(inputs["transforms_nkk"], np.float32)
    wm = np.asarray(inputs["W_mixer"], np.float32)

    in_maps = make_in_maps(x, wp, pm, tf, wm)
    nc = _get_nc()
    trace = bool(int(os.environ.get("KERNEL_TRACE", "0")))
    if trace:
        _ensure_ntff_hook()
    res = run_bass_kernel_spmd(nc, in_maps, list(range(8)), trace=trace)
    LAST_RESULTS = res
    outs = [r["out"] for r in res.results]
    full = np.empty((B, W, C), np.float32)
    for b in range(B):
        full[b] = outs[b] + outs[4 + b]
    return full
